# revision 40
# baseline (speedup 1.0000x reference)
"""Trainium2 Bass kernel for efficient-attention (nn_Attention_13280038880137), v2.

Model (per batch b):
  h = LayerNorm(x[b].T) * ln_w + ln_b          # (N, D), N=8192, D=512
  qkv = h @ w_qkv;  q,k,v -> (H=8, N, 64)
  q = softmax(q * 64**-.5, axis=tokens); k = softmax(k, axis=feat)
  C[h] = k[h].T @ v[h]                          # (64, 64)
  out = concat_h(q[h] @ C[h]) @ w_out + b_out   # (N, D) -> (D, N)

Under axon the dominant cost is the host<->device tunnel (~45-55 MB/s each
way, ~duplex); device compute is ~1 ms. So v2 optimizes bytes moved:

  - int8 on the wire both ways (4x fewer bytes than the f32 baseline, which
    also shipped x twice and 128 MB of donated zero output buffers):
      x: quantized host-side at per-token 127/absmax. LayerNorm is exactly
         scale-invariant per token, so the scale cancels on device with no
         dequant (only the eps weight shifts by ~1e-8 relative).
      y: quantized on device at per-(row, 512-token tile) absmax/127 by an
         ACT round-to-nearest int8 cast; the f32 scales are bitcast-packed
         into the last LNT*4 output columns (a separate tiny output costs a
         ~75 ms D2H round trip). Host dequantizes while reassembling.
  - Two pipelined launches, each 2 batches x 4 quarter-token shards across
    the 8 cores, so launch 2's x upload overlaps launch 1's y download on
    the duplex tunnel. All matmul operands are fp16 (f32 PSUM accumulate);
    LN stats and softmax sums stay f32.
  - The token-softmax normalizer Z=sum_n exp(q) and the context C=k~^T v
    need all 8192 tokens of a batch, so each batch's 4 quarter-token cores
    AllReduce a tiny packed (C|Z) buffer (128x516 f32, replica groups
    [[0,1,2,3],[4,5,6,7]]) mid-kernel; everything else is core-local.
  - Weights (ln_w folded into wq/wkv) are cached DEVICE-RESIDENT as sharded
    jax arrays keyed by a content hash, so warm calls ship only x.
  - The NEFF is launched through a module-cached jax.jit of shard_map around
    concourse.bass2jax._bass_exec_p - the same lowering
    bass_utils.run_bass_kernel_spmd uses under axon - minus its per-call
    retrace and its donated zero output buffers (this kernel writes every
    output element, so uninit custom_call results are fine).

Per-core numerics (mirrors the v1 design, extended to all 8 heads):
  - x int8 -> fp16 convert on DVE; LN stats via ones-matmul on PE;
    rstd = exp(-0.5*ln(var+eps)) on ACT (ACT table set stays {Exp, Ln});
    stats broadcast to [128,TN] via K=1 PE matmuls into PSUM;
    h = x*A - B on DVE in fp16.
  - q: feature-major matmul -> ACT Exp(scale=1/8) -> fp16 expq (persistent,
    4 MB); per-(head,dim) sum-of-exp partials via DVE reduce (no max
    subtraction: |q|/8 is small for LN'd inputs). Z sums never use ACT
    accum_out (loses mass on HW with PSUM input).
  - k,v: token-major matmuls (lhsT = h token slices); k: ACT Exp, DVE
    per-head sum/recip/scale (feature softmax over 64).
  - C: 4 head-pair PSUM banks accumulate k~^T v over all 32 token subtiles;
    each pair's diagonal [128,128] block is the payload. start=True clears a
    whole PSUM bank, so pairs live in separate banks.
  - C,Z packed to a DRAM bounce buffer, pair-AllReduce, read back; P = C *
    (PSC/Z) block-diagonal in fp16 (PSC=256 keeps fp16 attn well above
    denormals); pass 2: attn = P^T expq; y = wout^T attn, final ACT applies
    scale 1/PSC + b_out bias and casts fp16.

PSUM budget (16 KB/partition): pass1 = C pairs 8K + stats 2K + A|B bcast 4K
+ shared q/kv matmul bank 2K; pass2 = C pairs 8K + attn 2x2K + y 2x2K.
"""

import hashlib

import numpy as np
import jax

from jax.experimental.shard_map import shard_map  # same import bass2jax uses
from jax.sharding import Mesh, NamedSharding, PartitionSpec

import concourse.bass as bass
import concourse.bacc as bacc
import concourse.tile as tile
from concourse import mybir
from concourse.bass2jax import (
    _bass_exec_p,
    install_neuronx_cc_hook,
    partition_id_tensor,
)

F32 = mybir.dt.float32
F16 = mybir.dt.float16
I8 = mybir.dt.int8
AF = mybir.ActivationFunctionType
ALU = mybir.AluOpType

B = 4
D = 512
N = 8192
HEADS = 8
DH = 64
TN = 512            # token tile
DC = D // 128       # 4 feature chunks
NP = HEADS // 2     # 4 head pairs
NTOT = N // TN      # 16 token tiles per batch
# two pipelined launches x (2 batches x 4 quarter-token shards) so launch 2's
# x upload overlaps launch 1's y download on the duplex axon tunnel
LTOK = N // 4       # tokens per core per launch
LNT = LTOK // TN    # 4
SCALE = DH ** -0.5
EPS = 1e-5
PSC = 256.0         # context pre-scale for fp16 conditioning

TRACE = False
LAST_RESULT = None


def build_nc(has_lnb: bool):
    nc = bacc.Bacc(None, num_devices=8)
    # x ships as int8 (host scales by 127/absmax; LayerNorm is scale-invariant
    # so no device-side dequant scale is needed - only the eps weight shifts
    # by ~1e-8 relative, far below fp16 noise).
    x_d = nc.declare_dram_parameter("x", [DC, 128, LTOK], I8, isOutput=False)
    wq_d = nc.declare_dram_parameter("wq", [DC, 128, 512], F16, isOutput=False)
    wkv_d = nc.declare_dram_parameter("wkv", [DC, 128, 1024], F16, isOutput=False)
    wout_d = nc.declare_dram_parameter("wout", [DC, 128, D], F16, isOutput=False)
    bias_d = nc.declare_dram_parameter("bias", [DC, 128, 1], F32, isOutput=False)
    if has_lnb:
        qb_d = nc.declare_dram_parameter("qb", [DC, 128, 1], F32, isOutput=False)
        kvb_d = nc.declare_dram_parameter("kvb", [1, 1024], F16, isOutput=False)
    # y ships back as int8 + per-(row, 512-token tile) f32 scales (absmax/127),
    # bitcast-packed into the last NT*4 columns to avoid a second D2H pull
    out_d = nc.declare_dram_parameter(
        "out", [DC, 128, LTOK + LNT * 4], I8, isOutput=True
    )

    with tile.TileContext(nc) as tc:
        with (
            tc.tile_pool(name="singles", bufs=1) as singles,
            tc.tile_pool(name="persist", bufs=1) as persist,
            tc.tile_pool(name="psc", bufs=1, space=bass.MemorySpace.PSUM) as pscp,
        ):
            # ---- constants / weights ----
            wq_sb = singles.tile([128, DC, 512], F16)
            wkv_sb = singles.tile([128, DC, 1024], F16)
            wout_sb = singles.tile([128, DC, D], F16)
            bias_sb = singles.tile([128, DC], F32)
            for ci in range(DC):
                nc.sync.dma_start(out=wq_sb[:, ci, :], in_=wq_d[ci])
                nc.sync.dma_start(out=wkv_sb[:, ci, :], in_=wkv_d[ci])
                nc.sync.dma_start(out=wout_sb[:, ci, :], in_=wout_d[ci])
                nc.sync.dma_start(out=bias_sb[:, ci : ci + 1], in_=bias_d[ci])
            if has_lnb:
                qb_sb = singles.tile([128, DC], F32)
                kvb_sb = singles.tile([1, 1024], F16)
                for ci in range(DC):
                    nc.sync.dma_start(out=qb_sb[:, ci : ci + 1], in_=qb_d[ci])
                nc.sync.dma_start(out=kvb_sb[:], in_=kvb_d[:])

            ones_cf = singles.tile([128, 1], F32)
            ones_rf = singles.tile([1, 128], F32)
            eps_one = singles.tile([1, 1], F32)
            tiny_col = singles.tile([128, 1], F32)
            nc.vector.memset(ones_cf[:], 1.0)
            nc.vector.memset(ones_rf[:], 1.0)
            nc.vector.memset(eps_one[:], EPS)
            nc.vector.memset(tiny_col[:], 1e-30)
            ones_col = singles.tile([128, 1], F16)  # stats lhsT (K=128, M=1)
            ones_row = singles.tile([1, 128], F16)  # bcast lhsT (K=1, M=128)
            nc.vector.tensor_copy(ones_col[:], ones_cf[:])
            nc.vector.tensor_copy(ones_row[:], ones_rf[:])

            expq = persist.tile([128, NP, LTOK], F16)     # 2 MB persistent
            zq_parts = persist.tile([128, NP, LNT], F32)
            pbd = persist.tile([128, NP, 128], F16)       # P block-diag
            ps_c = [
                pscp.tile([128, 512], F32, tag=f"c{p}", name=f"ps_c{p}")
                for p in range(NP)
            ]

            # ---------------- pass 1 ----------------
            with (
                tc.tile_pool(name="xp", bufs=3) as xp,
                tc.tile_pool(name="sq", bufs=2) as sqp,
                tc.tile_pool(name="hp", bufs=2) as hp,
                tc.tile_pool(name="rows", bufs=2) as rows,
                tc.tile_pool(name="kvs", bufs=3) as kvs,
                tc.tile_pool(name="small", bufs=3) as small,
                tc.tile_pool(name="pss", bufs=1, space=bass.MemorySpace.PSUM) as pss,
                tc.tile_pool(name="psab", bufs=1, space=bass.MemorySpace.PSUM) as psab,
                tc.tile_pool(name="psmm", bufs=1, space=bass.MemorySpace.PSUM) as psmm,
            ):
                for t in range(LNT):
                    n0 = t * TN
                    x_i8 = xp.tile([128, DC, TN], I8, tag="xi8")
                    for ci in range(DC):
                        nc.sync.dma_start(
                            out=x_i8[:, ci, :], in_=x_d[ci, :, n0 : n0 + TN]
                        )
                    x_t = xp.tile([128, DC, TN], F16, tag="x")
                    xsq = sqp.tile([128, DC, TN], F16, tag="xsq")
                    for ci in range(DC):
                        nc.vector.tensor_copy(x_t[:, ci, :], x_i8[:, ci, :])
                        nc.vector.tensor_mul(
                            xsq[:, ci, :], x_t[:, ci, :], x_t[:, ci, :]
                        )
                    ps_s = pss.tile([1, TN], F32, tag="s")
                    for ci in range(DC):
                        nc.tensor.matmul(
                            ps_s[:], ones_col[:], x_t[:, ci, :],
                            start=(ci == 0), stop=(ci == DC - 1),
                        )
                    s_sb = rows.tile([1, TN], F32, tag="s_sb")
                    nc.scalar.copy(s_sb[:], ps_s[:])
                    ps_s2 = pss.tile([1, TN], F32, tag="s")
                    for ci in range(DC):
                        nc.tensor.matmul(
                            ps_s2[:], ones_col[:], xsq[:, ci, :],
                            start=(ci == 0), stop=(ci == DC - 1),
                        )
                    # var_raw = s2 - s^2/D ; rstd = exp(-.5*ln(var_raw/D+eps))
                    ssq = rows.tile([1, TN], F32, tag="ssq")
                    nc.vector.tensor_mul(ssq[:], s_sb[:], s_sb[:])
                    var_raw = rows.tile([1, TN], F32, tag="var")
                    nc.vector.scalar_tensor_tensor(
                        out=var_raw[:], in0=ssq[:], scalar=-1.0 / D, in1=ps_s2[:],
                        op0=ALU.mult, op1=ALU.add,
                    )
                    lnv = rows.tile([1, TN], F32, tag="lnv")
                    nc.scalar.activation(
                        out=lnv[:], in_=var_raw[:], func=AF.Ln,
                        scale=1.0 / D, bias=eps_one[:],
                    )
                    rstd = rows.tile([1, TN], F16, tag="rstd")
                    nc.scalar.activation(out=rstd[:], in_=lnv[:], func=AF.Exp, scale=-0.5)
                    mr = rows.tile([1, TN], F16, tag="mr")
                    nc.vector.scalar_tensor_tensor(
                        out=mr[:], in0=s_sb[:], scalar=1.0 / D, in1=rstd[:],
                        op0=ALU.mult, op1=ALU.mult,
                    )
                    # broadcast rstd (A) / mean*rstd (B) to [128, TN]
                    ab = psab.tile([128, 2 * TN], F32, tag="ab")
                    nc.tensor.matmul(
                        ab[:, 0:TN], ones_row[:], rstd[:], start=True, stop=True
                    )
                    nc.tensor.matmul(
                        ab[:, TN :], ones_row[:], mr[:], start=True, stop=True
                    )
                    # h = x*A - B  (fp16)
                    h = hp.tile([128, DC, TN], F16, tag="h")
                    for ci in range(DC):
                        nc.vector.tensor_mul(h[:, ci, :], x_t[:, ci, :], ab[:, 0:TN])
                        nc.vector.tensor_sub(h[:, ci, :], h[:, ci, :], ab[:, TN:])
                    # q: feature-major -> exp(q/8), Z partials
                    for jc in range(NP):
                        ps_q = psmm.tile([128, TN], F32, tag="mm")
                        for ci in range(DC):
                            nc.tensor.matmul(
                                ps_q[:],
                                wq_sb[:, ci, jc * 128 : jc * 128 + 128],
                                h[:, ci, :],
                                start=(ci == 0), stop=(ci == DC - 1),
                            )
                        if has_lnb:
                            nc.scalar.activation(
                                out=expq[:, jc, n0 : n0 + TN], in_=ps_q[:],
                                func=AF.Exp, scale=SCALE, bias=qb_sb[:, jc : jc + 1],
                            )
                        else:
                            nc.scalar.activation(
                                out=expq[:, jc, n0 : n0 + TN], in_=ps_q[:],
                                func=AF.Exp, scale=SCALE,
                            )
                    nc.vector.tensor_reduce(
                        zq_parts[:, :, t], expq[:, :, n0 : n0 + TN],
                        axis=mybir.AxisListType.X, op=ALU.add,
                    )
                    # k,v token-major; C accumulation per head pair
                    for ns in range(TN // 128):
                        tok = ns * 128
                        ps_k = psmm.tile([128, 512], F32, tag="mm")
                        for ci in range(DC):
                            nc.tensor.matmul(
                                ps_k[:],
                                h[:, ci, tok : tok + 128],
                                wkv_sb[:, ci, 0:512],
                                start=(ci == 0),
                                stop=(ci == DC - 1 and not has_lnb),
                            )
                        if has_lnb:
                            nc.tensor.matmul(
                                ps_k[:], ones_row[:], kvb_sb[:, 0:512],
                                start=False, stop=True,
                            )
                        ksm = kvs.tile([128, 512], F16, tag="ksm")
                        nc.scalar.activation(out=ksm[:], in_=ps_k[:], func=AF.Exp)
                        zk = small.tile([128, HEADS], F32, tag="zk")
                        nc.vector.tensor_reduce(
                            zk[:], ksm.rearrange("p (h e) -> p h e", h=HEADS),
                            axis=mybir.AxisListType.X, op=ALU.add,
                        )
                        zr = small.tile([128, HEADS], F32, tag="zr")
                        nc.vector.reciprocal(zr[:], zk[:])
                        ksr = kvs.tile([128, 512], F16, tag="ksr")
                        for hh in range(HEADS):
                            nc.vector.tensor_scalar_mul(
                                ksr[:, hh * DH : hh * DH + DH],
                                ksm[:, hh * DH : hh * DH + DH],
                                zr[:, hh : hh + 1],
                            )
                        ps_v = psmm.tile([128, 512], F32, tag="mm")
                        for ci in range(DC):
                            nc.tensor.matmul(
                                ps_v[:],
                                h[:, ci, tok : tok + 128],
                                wkv_sb[:, ci, 512:1024],
                                start=(ci == 0),
                                stop=(ci == DC - 1 and not has_lnb),
                            )
                        if has_lnb:
                            nc.tensor.matmul(
                                ps_v[:], ones_row[:], kvb_sb[:, 512:1024],
                                start=False, stop=True,
                            )
                        v_sb = kvs.tile([128, 512], F16, tag="v")
                        nc.vector.tensor_copy(v_sb[:], ps_v[:])
                        for p in range(NP):
                            nc.tensor.matmul(
                                ps_c[p][:],
                                ksr[:, p * 128 : p * 128 + 128],
                                v_sb[:],
                                start=(t == 0 and ns == 0),
                                stop=(t == LNT - 1 and ns == TN // 128 - 1),
                            )

            # ---- cross-core reduce of (C | Z), then P = C * (PSC/Z) ----
            with (
                tc.tile_pool(name="dramp", bufs=1, space="DRAM") as dramp,
                tc.tile_pool(name="czp", bufs=1) as czp,
            ):
                czin = dramp.tile([128, 516], F32)
                czout = dramp.tile([128, 516], F32)
                cz_sb = czp.tile([128, 516], F32)
                nc.vector.tensor_reduce(
                    cz_sb[:, 512:516], zq_parts[:], axis=mybir.AxisListType.X,
                    op=ALU.add,
                )
                for p in range(NP):
                    nc.scalar.copy(
                        cz_sb[:, p * 128 : p * 128 + 128],
                        ps_c[p][:, p * 128 : p * 128 + 128],
                    )
                nc.gpsimd.dma_start(czin[:], cz_sb[:])
                nc.gpsimd.collective_compute(
                    "AllReduce",
                    ALU.add,
                    replica_groups=[[0, 1, 2, 3], [4, 5, 6, 7]],
                    ins=[czin.opt()],
                    outs=[czout.opt()],
                )
                cfull = czp.tile([128, 516], F32)
                nc.gpsimd.dma_start(cfull[:], czout[:])
                rq = czp.tile([128, NP], F32)
                nc.vector.reciprocal(rq[:], cfull[:, 512:516])
                rqs = czp.tile([128, NP], F32)
                nc.scalar.activation(out=rqs[:], in_=rq[:], func=AF.Copy, scale=PSC)
                nc.vector.memset(pbd[:], 0.0)
                for p in range(NP):
                    base = p * 128
                    nc.vector.tensor_scalar_mul(
                        pbd[0:64, p, 0:64], cfull[0:64, base : base + 64],
                        rqs[0:64, p : p + 1],
                    )
                    nc.vector.tensor_scalar_mul(
                        pbd[64:128, p, 64:128], cfull[64:128, base + 64 : base + 128],
                        rqs[64:128, p : p + 1],
                    )

            # ---------------- pass 2 ----------------
            with (
                tc.tile_pool(name="attn", bufs=2) as attnp,
                tc.tile_pool(name="yp", bufs=3) as yp,
                tc.tile_pool(name="scp", bufs=1) as scp,
                tc.tile_pool(name="psa", bufs=2, space=bass.MemorySpace.PSUM) as psa,
                tc.tile_pool(name="psy", bufs=2, space=bass.MemorySpace.PSUM) as psy,
            ):
                sc_all = scp.tile([128, DC, LNT], F32)
                for t in range(LNT):
                    n0 = t * TN
                    attn_sb = attnp.tile([128, NP, TN], F16, tag="attn")
                    for jc in range(NP):
                        ps_at = psa.tile([128, TN], F32, tag="at")
                        nc.tensor.matmul(
                            ps_at[:], pbd[:, jc, :], expq[:, jc, n0 : n0 + TN],
                            start=True, stop=True,
                        )
                        nc.scalar.copy(attn_sb[:, jc, :], ps_at[:])
                    for mc in range(DC):
                        ps_y = psy.tile([128, TN], F32, tag="y")
                        for hc in range(NP):
                            nc.tensor.matmul(
                                ps_y[:],
                                wout_sb[:, hc, mc * 128 : mc * 128 + 128],
                                attn_sb[:, hc, :],
                                start=(hc == 0), stop=(hc == NP - 1),
                            )
                        y_sb = yp.tile([128, TN], F16, tag="y_sb")
                        nc.scalar.activation(
                            out=y_sb[:], in_=ps_y[:], func=AF.Identity,
                            scale=1.0 / PSC, bias=bias_sb[:, mc : mc + 1],
                        )
                        # per-row absmax -> scale; ACT int8 cast rounds-to-nearest
                        amax = yp.tile([128, 1], F32, tag="amax")
                        nc.vector.tensor_reduce(
                            amax[:], y_sb[:], axis=mybir.AxisListType.X,
                            op=ALU.max, apply_absolute_value=True,
                        )
                        nc.scalar.activation(
                            out=sc_all[:, mc, t : t + 1], in_=amax[:],
                            func=AF.Identity, scale=1.0 / 127.0, bias=tiny_col[:],
                        )
                        rs = yp.tile([128, 1], F32, tag="rs")
                        nc.vector.reciprocal(rs[:], sc_all[:, mc, t : t + 1])
                        q_sb = yp.tile([128, TN], I8, tag="q_sb")
                        nc.scalar.activation(
                            out=q_sb[:], in_=y_sb[:], func=AF.Copy, scale=rs[:]
                        )
                        nc.sync.dma_start(
                            out=out_d[mc, :, n0 : n0 + TN], in_=q_sb[:]
                        )
                for mc in range(DC):
                    nc.sync.dma_start(
                        out=out_d[mc, :, LTOK : LTOK + LNT * 4],
                        in_=sc_all[:, mc, :].bitcast(I8),
                    )
    nc.finalize()
    return nc


def _make_runner(nc):
    install_neuronx_cc_hook()
    in_names, out_names, out_avals = [], [], []
    partition_name = nc.partition_id_tensor.name if nc.partition_id_tensor else None
    for alloc in nc.m.functions[0].allocations:
        if not isinstance(alloc, mybir.MemoryLocationSet):
            continue
        name = alloc.memorylocations[0].name
        if alloc.kind == "ExternalInput":
            if name != partition_name:
                in_names.append(name)
        elif alloc.kind == "ExternalOutput":
            out_names.append(name)
            out_avals.append(
                jax.core.ShapedArray(
                    tuple(alloc.tensor_shape), mybir.dt.np(alloc.dtype)
                )
            )
    bind_names = tuple(in_names + ([partition_name] if partition_name else []))

    def _body(*args):
        operands = list(args)
        if partition_name is not None:
            operands.append(partition_id_tensor())
        outs = _bass_exec_p.bind(
            *operands,
            out_avals=tuple(out_avals),
            in_names=bind_names,
            out_names=tuple(out_names),
            lowering_input_output_aliases=(),
            sim_require_finite=True,
            sim_require_nnan=True,
            nc=nc,
        )
        return tuple(outs)

    devices = jax.devices()[:8]
    mesh = Mesh(np.asarray(devices), ("core",))
    fn = jax.jit(
        shard_map(
            _body,
            mesh=mesh,
            in_specs=(PartitionSpec("core"),) * len(in_names),
            out_specs=(PartitionSpec("core"),) * len(out_names),
            check_rep=False,
        )
    )
    return fn, mesh, in_names, out_names


_ST = {}


def _replicate8(a):
    """Stack 8 identical per-core copies along axis 0 (for sharded transfer)."""
    return np.ascontiguousarray(
        np.broadcast_to(a[None], (8,) + a.shape)
    ).reshape((8 * a.shape[0],) + a.shape[1:])


def _reset_backend():
    """Tear down the (possibly wedged) axon PJRT client so the next jax call
    reconnects. Used to retry after 'worker hung up' / 'mesh desynced'."""
    import jax._src.xla_bridge as xb

    _ST.clear()
    try:
        jax.clear_caches()
    except Exception:
        pass
    try:
        xb._clear_backends()
    except Exception:
        pass


def _get_state(has_lnb):
    st = _ST.get(has_lnb)
    if st is None:
        # canary: exercise all 8 devices with a trivial sharded op before
        # loading the heavy collective program
        devs = jax.devices()[:8]
        mesh0 = Mesh(np.asarray(devs), ("core",))
        canary = jax.device_put(
            np.zeros((8, 8), np.float32), NamedSharding(mesh0, PartitionSpec("core"))
        )
        (canary + 1.0).block_until_ready()
        nc = build_nc(has_lnb)
        fn, mesh, in_names, out_names = _make_runner(nc)
        st = {
            "fn": fn,
            "mesh": mesh,
            "in_names": in_names,
            "out_names": out_names,
            "shard": NamedSharding(mesh, PartitionSpec("core")),
            "dev2idx": {d: i for i, d in enumerate(mesh.devices.flat)},
            "wkey": None,
        }
        _ST[has_lnb] = st
    return st


def kernel(x, ln_w, ln_b, w_qkv, w_out, b_out):
    x = np.asarray(x, dtype=np.float32)
    ln_w = np.asarray(ln_w, dtype=np.float32)
    ln_b = np.asarray(ln_b, dtype=np.float32)
    w_qkv = np.asarray(w_qkv, dtype=np.float32)
    w_out = np.asarray(w_out, dtype=np.float32)
    b_out = np.asarray(b_out, dtype=np.float32)
    assert x.shape == (B, D, N)

    has_lnb = bool(np.any(ln_b != 0.0))
    hsh = hashlib.blake2b(digest_size=16)
    for a in (ln_w, ln_b, w_qkv, w_out, b_out):
        hsh.update(np.ascontiguousarray(a).tobytes())
    wkey = hsh.digest()

    # x -> int8 at per-token 127/absmax (per-token scale cancels inside the
    # device LayerNorm, so no dequant anywhere; per-token beats a global
    # scale by ~1.4x in quantization noise)
    tsc = 127.0 / np.maximum(
        np.maximum(x.max(axis=1), -x.min(axis=1)), 1e-30
    )  # [B, N]

    # The axon tunnel intermittently drops on the first heavy launch after
    # other jax work in the same process ("worker hung up"/"mesh desynced");
    # reset the client and retry.
    last = None
    for _ in range(3):
        try:
            st = _get_state(has_lnb)
            _put_weights(st, wkey, has_lnb, ln_w, ln_b, w_qkv, w_out, b_out)
            return _run(st, x, tsc)
        except RuntimeError as e:
            last = e
            _reset_backend()
    raise last


def _put_weights(st, wkey, has_lnb, ln_w, ln_b, w_qkv, w_out, b_out):
    if st["wkey"] == wkey:
        return
    wq = (w_qkv[:, 0:512] * ln_w[:, None]).astype(np.float16)
    wk = w_qkv[:, 512:1024] * ln_w[:, None]
    wv = w_qkv[:, 1024:1536] * ln_w[:, None]
    wkv = np.concatenate([wk, wv], axis=1).astype(np.float16)
    host = {
        "wq": _replicate8(wq.reshape(DC, 128, 512)),
        "wkv": _replicate8(wkv.reshape(DC, 128, 1024)),
        "wout": _replicate8(w_out.astype(np.float16).reshape(DC, 128, D)),
        "bias": _replicate8(b_out.reshape(DC, 128, 1)),
    }
    if has_lnb:
        qb = (SCALE * (ln_b @ (w_qkv[:, 0:512] * ln_w[:, None]))).astype(np.float32)
        kvb = (ln_b @ np.concatenate([wk, wv], axis=1)).astype(np.float16)
        host["qb"] = _replicate8(qb.reshape(DC, 128, 1))
        host["kvb"] = _replicate8(kvb.reshape(1, 1024))
    st["devw"] = {k: jax.device_put(v, st["shard"]) for k, v in host.items()}
    for a in st["devw"].values():
        a.block_until_ready()
    st["wkey"] = wkey


def _run(st, x, tsc):
    res = np.empty((B, D, N), np.float32)
    out_idx = st["out_names"].index("out")

    # Both launches are dispatched up front (jax dispatch and device_put are
    # async), so launch 2's x upload pipelines behind launch 1's exec and
    # overlaps launch 1's y download on the duplex tunnel. All jax calls stay
    # on this one thread: concurrent multi-thread entry into the axon client
    # intermittently drops the tunnel ("notify failed ... worker hung up").
    # The very first launch after other jax work (e.g. a jitted reference in
    # the same process) runs fully synchronously: a cold pipelined burst on a
    # busy terminal intermittently drops the tunnel too.
    sync = not st.get("warmed", False)
    st["warmed"] = True
    ogs = []
    for launch in range(2):
        xg = np.empty((8 * DC, 128, LTOK), np.int8)
        xv = xg.reshape(8, DC, 128, LTOK)
        tmp = np.empty((DC, 128, LTOK), np.float32)
        for c in range(8):
            b, q = 2 * launch + (c >> 2), c & 3
            sl = slice(q * LTOK, (q + 1) * LTOK)
            np.multiply(x[b].reshape(DC, 128, N)[:, :, sl], tsc[b, sl], out=tmp)
            np.rint(tmp, out=tmp)
            xv[c] = tmp
        xd = jax.device_put(xg, st["shard"])
        if sync:
            xd.block_until_ready()
        args = []
        for name in st["in_names"]:
            args.append(xd if name == "x" else st["devw"][name])
        og = st["fn"](*args)[out_idx]
        if sync:
            og.block_until_ready()
        ogs.append(og)

    for launch in range(2):
        o = np.asarray(ogs[launch]).reshape(8, DC, 128, LTOK + LNT * 4)
        sc = o[:, :, :, LTOK:].copy().view(np.float32)  # [8, DC, 128, LNT]
        y = o[:, :, :, :LTOK].reshape(8, DC, 128, LNT, TN)
        for c in range(8):
            b, q = 2 * launch + (c >> 2), c & 3
            rv = res[b].reshape(DC, 128, NTOT, TN)
            rv[:, :, q * LNT : (q + 1) * LNT, :] = y[c] * sc[c][:, :, :, None]
    return res


# revision 54
# speedup vs baseline: 1.2543x; 1.2543x over previous
"""Trainium2 Bass kernel for efficient-attention (nn_Attention_13280038880137), v2.

Model (per batch b):
  h = LayerNorm(x[b].T) * ln_w + ln_b          # (N, D), N=8192, D=512
  qkv = h @ w_qkv;  q,k,v -> (H=8, N, 64)
  q = softmax(q * 64**-.5, axis=tokens); k = softmax(k, axis=feat)
  C[h] = k[h].T @ v[h]                          # (64, 64)
  out = concat_h(q[h] @ C[h]) @ w_out + b_out   # (N, D) -> (D, N)

Under axon the dominant cost is the host<->device tunnel (~45-55 MB/s each
way, ~duplex); device compute is ~1 ms. So v2 optimizes bytes moved:

  - int8 on the wire both ways (4x fewer bytes than the f32 baseline, which
    also shipped x twice and 128 MB of donated zero output buffers):
      x: quantized host-side at per-token 127/absmax. LayerNorm is exactly
         scale-invariant per token, so the scale cancels on device with no
         dequant (only the eps weight shifts by ~1e-8 relative).
      y: quantized on device at per-(row, 512-token tile) absmax/127 by an
         ACT round-to-nearest int8 cast; the f32 scales are bitcast-packed
         into the last LNT*4 output columns (a separate tiny output costs a
         ~75 ms D2H round trip). Host dequantizes while reassembling.
  - One launch: 4 batches x 2 half-token shards across the 8 cores. (Split
    pipelined launches were measured slower: each extra launch costs
    ~60-100 ms of dispatch + per-transfer overhead and the client does not
    actually overlap the streams. NLAUNCH is kept as a knob.) All matmul
    operands are fp16 (f32 PSUM accumulate); LN stats and softmax sums
    stay f32.
  - The token-softmax normalizer Z=sum_n exp(q) and the context C=k~^T v
    need all 8192 tokens of a batch, so each batch's token-shard cores
    AllReduce a tiny packed (C|Z) buffer (128x516 f32, replica groups
    CCGROUPS) mid-kernel; everything else is core-local.
  - Weights (ln_w folded into wq/wkv) are cached DEVICE-RESIDENT as sharded
    jax arrays keyed by a content hash, so warm calls ship only x.
  - The NEFF is launched through a module-cached jax.jit of shard_map around
    concourse.bass2jax._bass_exec_p - the same lowering
    bass_utils.run_bass_kernel_spmd uses under axon - minus its per-call
    retrace and its donated zero output buffers (this kernel writes every
    output element, so uninit custom_call results are fine).

Per-core numerics (mirrors the v1 design, extended to all 8 heads):
  - x int8 -> fp16 convert on DVE; LN stats via ones-matmul on PE;
    rstd = exp(-0.5*ln(var+eps)) on ACT (ACT table set stays {Exp, Ln});
    stats broadcast to [128,TN] via K=1 PE matmuls into PSUM;
    h = x*A - B on DVE in fp16.
  - q: feature-major matmul -> ACT Exp(scale=1/8) -> fp16 expq (persistent,
    4 MB); per-(head,dim) sum-of-exp partials via DVE reduce (no max
    subtraction: |q|/8 is small for LN'd inputs). Z sums never use ACT
    accum_out (loses mass on HW with PSUM input).
  - k,v: token-major matmuls (lhsT = h token slices); k: ACT Exp, DVE
    per-head sum/recip/scale (feature softmax over 64).
  - C: 4 head-pair PSUM banks accumulate k~^T v over all 32 token subtiles;
    each pair's diagonal [128,128] block is the payload. start=True clears a
    whole PSUM bank, so pairs live in separate banks.
  - C,Z packed to a DRAM bounce buffer, pair-AllReduce, read back; P = C *
    (PSC/Z) block-diagonal in fp16 (PSC=256 keeps fp16 attn well above
    denormals); pass 2: attn = P^T expq; y = wout^T attn, final ACT applies
    scale 1/PSC + b_out bias and casts fp16.

PSUM budget (16 KB/partition): pass1 = C pairs 8K + stats 2K + A|B bcast 4K
+ shared q/kv matmul bank 2K; pass2 = C pairs 8K + attn 2x2K + y 2x2K.
"""

import hashlib

import numpy as np
import jax

from jax.experimental.shard_map import shard_map  # same import bass2jax uses
from jax.sharding import Mesh, NamedSharding, PartitionSpec

import concourse.bass as bass
import concourse.bacc as bacc
import concourse.tile as tile
from concourse import mybir
from concourse.bass2jax import (
    _bass_exec_p,
    install_neuronx_cc_hook,
    partition_id_tensor,
)

F32 = mybir.dt.float32
F16 = mybir.dt.float16
I8 = mybir.dt.int8
AF = mybir.ActivationFunctionType
ALU = mybir.AluOpType

B = 4
D = 512
N = 8192
HEADS = 8
DH = 64
TN = 512            # token tile
DC = D // 128       # 4 feature chunks
NP = HEADS // 2     # 4 head pairs
NTOT = N // TN      # 16 token tiles per batch
# NLAUNCH pipelined launches so launch k+1's x upload overlaps launch k's y
# download on the duplex axon tunnel. Each launch covers B/NLAUNCH batches,
# each batch split over 8*NLAUNCH/B cores by token range; the (C|Z) stats
# AllReduce groups the cores of one batch.
NLAUNCH = 1
BPL = B // NLAUNCH          # batches per launch
CPB = 8 // BPL              # cores per batch
LTOK = N // CPB             # tokens per core per launch
LNT = LTOK // TN
CCGROUPS = [[g * CPB + i for i in range(CPB)] for g in range(BPL)]
SCALE = DH ** -0.5
EPS = 1e-5
PSC = 256.0         # context pre-scale for fp16 conditioning

TRACE = False
LAST_RESULT = None


def build_nc(has_lnb: bool):
    nc = bacc.Bacc(None, num_devices=8)
    # x ships as int8 (host scales by 127/absmax; LayerNorm is scale-invariant
    # so no device-side dequant scale is needed - only the eps weight shifts
    # by ~1e-8 relative, far below fp16 noise).
    x_d = nc.declare_dram_parameter("x", [DC, 128, LTOK], I8, isOutput=False)
    wq_d = nc.declare_dram_parameter("wq", [DC, 128, 512], F16, isOutput=False)
    wkv_d = nc.declare_dram_parameter("wkv", [DC, 128, 1024], F16, isOutput=False)
    wout_d = nc.declare_dram_parameter("wout", [DC, 128, D], F16, isOutput=False)
    bias_d = nc.declare_dram_parameter("bias", [DC, 128, 1], F32, isOutput=False)
    if has_lnb:
        qb_d = nc.declare_dram_parameter("qb", [DC, 128, 1], F32, isOutput=False)
        kvb_d = nc.declare_dram_parameter("kvb", [1, 1024], F16, isOutput=False)
    # y ships back as int8 + per-(row, 512-token tile) f32 scales (absmax/127),
    # bitcast-packed into the last NT*4 columns to avoid a second D2H pull
    out_d = nc.declare_dram_parameter(
        "out", [DC, 128, LTOK + LNT * 4], I8, isOutput=True
    )

    with tile.TileContext(nc) as tc:
        with (
            tc.tile_pool(name="singles", bufs=1) as singles,
            tc.tile_pool(name="persist", bufs=1) as persist,
            tc.tile_pool(name="psc", bufs=1, space=bass.MemorySpace.PSUM) as pscp,
        ):
            # ---- constants / weights ----
            wq_sb = singles.tile([128, DC, 512], F16)
            wkv_sb = singles.tile([128, DC, 1024], F16)
            wout_sb = singles.tile([128, DC, D], F16)
            bias_sb = singles.tile([128, DC], F32)
            for ci in range(DC):
                nc.sync.dma_start(out=wq_sb[:, ci, :], in_=wq_d[ci])
                nc.sync.dma_start(out=wkv_sb[:, ci, :], in_=wkv_d[ci])
                nc.sync.dma_start(out=wout_sb[:, ci, :], in_=wout_d[ci])
                nc.sync.dma_start(out=bias_sb[:, ci : ci + 1], in_=bias_d[ci])
            if has_lnb:
                qb_sb = singles.tile([128, DC], F32)
                kvb_sb = singles.tile([1, 1024], F16)
                for ci in range(DC):
                    nc.sync.dma_start(out=qb_sb[:, ci : ci + 1], in_=qb_d[ci])
                nc.sync.dma_start(out=kvb_sb[:], in_=kvb_d[:])

            ones_cf = singles.tile([128, 1], F32)
            ones_rf = singles.tile([1, 128], F32)
            eps_one = singles.tile([1, 1], F32)
            tiny_col = singles.tile([128, 1], F32)
            nc.vector.memset(ones_cf[:], 1.0)
            nc.vector.memset(ones_rf[:], 1.0)
            nc.vector.memset(eps_one[:], EPS)
            nc.vector.memset(tiny_col[:], 1e-30)
            ones_col = singles.tile([128, 1], F16)  # stats lhsT (K=128, M=1)
            ones_row = singles.tile([1, 128], F16)  # bcast lhsT (K=1, M=128)
            nc.vector.tensor_copy(ones_col[:], ones_cf[:])
            nc.vector.tensor_copy(ones_row[:], ones_rf[:])

            expq = persist.tile([128, NP, LTOK], F16)     # 2 MB persistent
            zq_parts = persist.tile([128, NP, LNT], F32)
            pbd = persist.tile([128, NP, 128], F16)       # P block-diag
            ps_c = [
                pscp.tile([128, 512], F32, tag=f"c{p}", name=f"ps_c{p}")
                for p in range(NP)
            ]

            # ---------------- pass 1 ----------------
            with (
                tc.tile_pool(name="xp", bufs=3) as xp,
                tc.tile_pool(name="sq", bufs=2) as sqp,
                tc.tile_pool(name="hp", bufs=2) as hp,
                tc.tile_pool(name="rows", bufs=2) as rows,
                tc.tile_pool(name="kvs", bufs=3) as kvs,
                tc.tile_pool(name="small", bufs=3) as small,
                tc.tile_pool(name="pss", bufs=1, space=bass.MemorySpace.PSUM) as pss,
                tc.tile_pool(name="psab", bufs=1, space=bass.MemorySpace.PSUM) as psab,
                tc.tile_pool(name="psmm", bufs=1, space=bass.MemorySpace.PSUM) as psmm,
            ):
                for t in range(LNT):
                    n0 = t * TN
                    x_i8 = xp.tile([128, DC, TN], I8, tag="xi8")
                    for ci in range(DC):
                        nc.sync.dma_start(
                            out=x_i8[:, ci, :], in_=x_d[ci, :, n0 : n0 + TN]
                        )
                    x_t = xp.tile([128, DC, TN], F16, tag="x")
                    xsq = sqp.tile([128, DC, TN], F16, tag="xsq")
                    for ci in range(DC):
                        nc.vector.tensor_copy(x_t[:, ci, :], x_i8[:, ci, :])
                        nc.vector.tensor_mul(
                            xsq[:, ci, :], x_t[:, ci, :], x_t[:, ci, :]
                        )
                    ps_s = pss.tile([1, TN], F32, tag="s")
                    for ci in range(DC):
                        nc.tensor.matmul(
                            ps_s[:], ones_col[:], x_t[:, ci, :],
                            start=(ci == 0), stop=(ci == DC - 1),
                        )
                    s_sb = rows.tile([1, TN], F32, tag="s_sb")
                    nc.scalar.copy(s_sb[:], ps_s[:])
                    ps_s2 = pss.tile([1, TN], F32, tag="s")
                    for ci in range(DC):
                        nc.tensor.matmul(
                            ps_s2[:], ones_col[:], xsq[:, ci, :],
                            start=(ci == 0), stop=(ci == DC - 1),
                        )
                    # var_raw = s2 - s^2/D ; rstd = exp(-.5*ln(var_raw/D+eps))
                    ssq = rows.tile([1, TN], F32, tag="ssq")
                    nc.vector.tensor_mul(ssq[:], s_sb[:], s_sb[:])
                    var_raw = rows.tile([1, TN], F32, tag="var")
                    nc.vector.scalar_tensor_tensor(
                        out=var_raw[:], in0=ssq[:], scalar=-1.0 / D, in1=ps_s2[:],
                        op0=ALU.mult, op1=ALU.add,
                    )
                    lnv = rows.tile([1, TN], F32, tag="lnv")
                    nc.scalar.activation(
                        out=lnv[:], in_=var_raw[:], func=AF.Ln,
                        scale=1.0 / D, bias=eps_one[:],
                    )
                    rstd = rows.tile([1, TN], F16, tag="rstd")
                    nc.scalar.activation(out=rstd[:], in_=lnv[:], func=AF.Exp, scale=-0.5)
                    mr = rows.tile([1, TN], F16, tag="mr")
                    nc.vector.scalar_tensor_tensor(
                        out=mr[:], in0=s_sb[:], scalar=1.0 / D, in1=rstd[:],
                        op0=ALU.mult, op1=ALU.mult,
                    )
                    # broadcast rstd (A) / mean*rstd (B) to [128, TN]
                    ab = psab.tile([128, 2 * TN], F32, tag="ab")
                    nc.tensor.matmul(
                        ab[:, 0:TN], ones_row[:], rstd[:], start=True, stop=True
                    )
                    nc.tensor.matmul(
                        ab[:, TN :], ones_row[:], mr[:], start=True, stop=True
                    )
                    # h = x*A - B  (fp16)
                    h = hp.tile([128, DC, TN], F16, tag="h")
                    for ci in range(DC):
                        nc.vector.tensor_mul(h[:, ci, :], x_t[:, ci, :], ab[:, 0:TN])
                        nc.vector.tensor_sub(h[:, ci, :], h[:, ci, :], ab[:, TN:])
                    # q: feature-major -> exp(q/8), Z partials
                    for jc in range(NP):
                        ps_q = psmm.tile([128, TN], F32, tag="mm")
                        for ci in range(DC):
                            nc.tensor.matmul(
                                ps_q[:],
                                wq_sb[:, ci, jc * 128 : jc * 128 + 128],
                                h[:, ci, :],
                                start=(ci == 0), stop=(ci == DC - 1),
                            )
                        if has_lnb:
                            nc.scalar.activation(
                                out=expq[:, jc, n0 : n0 + TN], in_=ps_q[:],
                                func=AF.Exp, scale=SCALE, bias=qb_sb[:, jc : jc + 1],
                            )
                        else:
                            nc.scalar.activation(
                                out=expq[:, jc, n0 : n0 + TN], in_=ps_q[:],
                                func=AF.Exp, scale=SCALE,
                            )
                    nc.vector.tensor_reduce(
                        zq_parts[:, :, t], expq[:, :, n0 : n0 + TN],
                        axis=mybir.AxisListType.X, op=ALU.add,
                    )
                    # k,v token-major; C accumulation per head pair
                    for ns in range(TN // 128):
                        tok = ns * 128
                        ps_k = psmm.tile([128, 512], F32, tag="mm")
                        for ci in range(DC):
                            nc.tensor.matmul(
                                ps_k[:],
                                h[:, ci, tok : tok + 128],
                                wkv_sb[:, ci, 0:512],
                                start=(ci == 0),
                                stop=(ci == DC - 1 and not has_lnb),
                            )
                        if has_lnb:
                            nc.tensor.matmul(
                                ps_k[:], ones_row[:], kvb_sb[:, 0:512],
                                start=False, stop=True,
                            )
                        ksm = kvs.tile([128, 512], F16, tag="ksm")
                        nc.scalar.activation(out=ksm[:], in_=ps_k[:], func=AF.Exp)
                        zk = small.tile([128, HEADS], F32, tag="zk")
                        nc.vector.tensor_reduce(
                            zk[:], ksm.rearrange("p (h e) -> p h e", h=HEADS),
                            axis=mybir.AxisListType.X, op=ALU.add,
                        )
                        zr = small.tile([128, HEADS], F32, tag="zr")
                        nc.vector.reciprocal(zr[:], zk[:])
                        ksr = kvs.tile([128, 512], F16, tag="ksr")
                        for hh in range(HEADS):
                            nc.vector.tensor_scalar_mul(
                                ksr[:, hh * DH : hh * DH + DH],
                                ksm[:, hh * DH : hh * DH + DH],
                                zr[:, hh : hh + 1],
                            )
                        ps_v = psmm.tile([128, 512], F32, tag="mm")
                        for ci in range(DC):
                            nc.tensor.matmul(
                                ps_v[:],
                                h[:, ci, tok : tok + 128],
                                wkv_sb[:, ci, 512:1024],
                                start=(ci == 0),
                                stop=(ci == DC - 1 and not has_lnb),
                            )
                        if has_lnb:
                            nc.tensor.matmul(
                                ps_v[:], ones_row[:], kvb_sb[:, 512:1024],
                                start=False, stop=True,
                            )
                        v_sb = kvs.tile([128, 512], F16, tag="v")
                        nc.vector.tensor_copy(v_sb[:], ps_v[:])
                        for p in range(NP):
                            nc.tensor.matmul(
                                ps_c[p][:],
                                ksr[:, p * 128 : p * 128 + 128],
                                v_sb[:],
                                start=(t == 0 and ns == 0),
                                stop=(t == LNT - 1 and ns == TN // 128 - 1),
                            )

            # ---- cross-core reduce of (C | Z), then P = C * (PSC/Z) ----
            with (
                tc.tile_pool(name="dramp", bufs=1, space="DRAM") as dramp,
                tc.tile_pool(name="czp", bufs=1) as czp,
            ):
                czin = dramp.tile([128, 516], F32)
                czout = dramp.tile([128, 516], F32)
                cz_sb = czp.tile([128, 516], F32)
                nc.vector.tensor_reduce(
                    cz_sb[:, 512:516], zq_parts[:], axis=mybir.AxisListType.X,
                    op=ALU.add,
                )
                for p in range(NP):
                    nc.scalar.copy(
                        cz_sb[:, p * 128 : p * 128 + 128],
                        ps_c[p][:, p * 128 : p * 128 + 128],
                    )
                nc.gpsimd.dma_start(czin[:], cz_sb[:])
                nc.gpsimd.collective_compute(
                    "AllReduce",
                    ALU.add,
                    replica_groups=CCGROUPS,
                    ins=[czin.opt()],
                    outs=[czout.opt()],
                )
                cfull = czp.tile([128, 516], F32)
                nc.gpsimd.dma_start(cfull[:], czout[:])
                rq = czp.tile([128, NP], F32)
                nc.vector.reciprocal(rq[:], cfull[:, 512:516])
                rqs = czp.tile([128, NP], F32)
                nc.scalar.activation(out=rqs[:], in_=rq[:], func=AF.Copy, scale=PSC)
                nc.vector.memset(pbd[:], 0.0)
                for p in range(NP):
                    base = p * 128
                    nc.vector.tensor_scalar_mul(
                        pbd[0:64, p, 0:64], cfull[0:64, base : base + 64],
                        rqs[0:64, p : p + 1],
                    )
                    nc.vector.tensor_scalar_mul(
                        pbd[64:128, p, 64:128], cfull[64:128, base + 64 : base + 128],
                        rqs[64:128, p : p + 1],
                    )

            # ---------------- pass 2 ----------------
            with (
                tc.tile_pool(name="attn", bufs=2) as attnp,
                tc.tile_pool(name="yp", bufs=3) as yp,
                tc.tile_pool(name="scp", bufs=1) as scp,
                tc.tile_pool(name="psa", bufs=2, space=bass.MemorySpace.PSUM) as psa,
                tc.tile_pool(name="psy", bufs=2, space=bass.MemorySpace.PSUM) as psy,
            ):
                sc_all = scp.tile([128, DC, LNT], F32)
                for t in range(LNT):
                    n0 = t * TN
                    attn_sb = attnp.tile([128, NP, TN], F16, tag="attn")
                    for jc in range(NP):
                        ps_at = psa.tile([128, TN], F32, tag="at")
                        nc.tensor.matmul(
                            ps_at[:], pbd[:, jc, :], expq[:, jc, n0 : n0 + TN],
                            start=True, stop=True,
                        )
                        nc.scalar.copy(attn_sb[:, jc, :], ps_at[:])
                    for mc in range(DC):
                        ps_y = psy.tile([128, TN], F32, tag="y")
                        for hc in range(NP):
                            nc.tensor.matmul(
                                ps_y[:],
                                wout_sb[:, hc, mc * 128 : mc * 128 + 128],
                                attn_sb[:, hc, :],
                                start=(hc == 0), stop=(hc == NP - 1),
                            )
                        y_sb = yp.tile([128, TN], F16, tag="y_sb")
                        nc.scalar.activation(
                            out=y_sb[:], in_=ps_y[:], func=AF.Identity,
                            scale=1.0 / PSC, bias=bias_sb[:, mc : mc + 1],
                        )
                        # per-row absmax -> scale; ACT int8 cast rounds-to-nearest
                        amax = yp.tile([128, 1], F32, tag="amax")
                        nc.vector.tensor_reduce(
                            amax[:], y_sb[:], axis=mybir.AxisListType.X,
                            op=ALU.max, apply_absolute_value=True,
                        )
                        nc.scalar.activation(
                            out=sc_all[:, mc, t : t + 1], in_=amax[:],
                            func=AF.Identity, scale=1.0 / 127.0, bias=tiny_col[:],
                        )
                        rs = yp.tile([128, 1], F32, tag="rs")
                        nc.vector.reciprocal(rs[:], sc_all[:, mc, t : t + 1])
                        q_sb = yp.tile([128, TN], I8, tag="q_sb")
                        nc.scalar.activation(
                            out=q_sb[:], in_=y_sb[:], func=AF.Copy, scale=rs[:]
                        )
                        nc.sync.dma_start(
                            out=out_d[mc, :, n0 : n0 + TN], in_=q_sb[:]
                        )
                for mc in range(DC):
                    nc.sync.dma_start(
                        out=out_d[mc, :, LTOK : LTOK + LNT * 4],
                        in_=sc_all[:, mc, :].bitcast(I8),
                    )
    nc.finalize()
    return nc


def _make_runner(nc):
    install_neuronx_cc_hook()
    in_names, out_names, out_avals = [], [], []
    partition_name = nc.partition_id_tensor.name if nc.partition_id_tensor else None
    for alloc in nc.m.functions[0].allocations:
        if not isinstance(alloc, mybir.MemoryLocationSet):
            continue
        name = alloc.memorylocations[0].name
        if alloc.kind == "ExternalInput":
            if name != partition_name:
                in_names.append(name)
        elif alloc.kind == "ExternalOutput":
            out_names.append(name)
            out_avals.append(
                jax.core.ShapedArray(
                    tuple(alloc.tensor_shape), mybir.dt.np(alloc.dtype)
                )
            )
    bind_names = tuple(in_names + ([partition_name] if partition_name else []))

    def _body(*args):
        operands = list(args)
        if partition_name is not None:
            operands.append(partition_id_tensor())
        outs = _bass_exec_p.bind(
            *operands,
            out_avals=tuple(out_avals),
            in_names=bind_names,
            out_names=tuple(out_names),
            lowering_input_output_aliases=(),
            sim_require_finite=True,
            sim_require_nnan=True,
            nc=nc,
        )
        return tuple(outs)

    devices = jax.devices()[:8]
    mesh = Mesh(np.asarray(devices), ("core",))
    mapped = shard_map(
        _body,
        mesh=mesh,
        in_specs=(PartitionSpec("core"),) * len(in_names),
        out_specs=(PartitionSpec("core"),) * len(out_names),
        check_rep=False,
    )
    # (An AOT fast_dispatch_compile variant - ordered bass effect suppressed,
    # C++ fast dispatch - was measured at parity: the effect-token cost is
    # negligible next to the per-launch RPC latency. Plain jit is kept as the
    # battle-tested path.)
    fn = jax.jit(mapped)
    return fn, mesh, in_names, out_names


_ST = {}


def _replicate8(a):
    """Stack 8 identical per-core copies along axis 0 (for sharded transfer)."""
    return np.ascontiguousarray(
        np.broadcast_to(a[None], (8,) + a.shape)
    ).reshape((8 * a.shape[0],) + a.shape[1:])


def _reset_backend():
    """Tear down the (possibly wedged) axon PJRT client so the next jax call
    reconnects. Used to retry after 'worker hung up' / 'mesh desynced'."""
    import jax._src.xla_bridge as xb

    _ST.clear()
    try:
        jax.clear_caches()
    except Exception:
        pass
    try:
        xb._clear_backends()
    except Exception:
        pass


def _get_state(has_lnb):
    st = _ST.get(has_lnb)
    if st is None:
        # canary: exercise all 8 devices with a trivial sharded op before
        # loading the heavy collective program
        devs = jax.devices()[:8]
        mesh0 = Mesh(np.asarray(devs), ("core",))
        canary = jax.device_put(
            np.zeros((8, 8), np.float32), NamedSharding(mesh0, PartitionSpec("core"))
        )
        (canary + 1.0).block_until_ready()
        nc = build_nc(has_lnb)
        fn, mesh, in_names, out_names = _make_runner(nc)
        st = {
            "fn": fn,
            "mesh": mesh,
            "in_names": in_names,
            "out_names": out_names,
            "shard": NamedSharding(mesh, PartitionSpec("core")),
            "dev2idx": {d: i for i, d in enumerate(mesh.devices.flat)},
            "wkey": None,
        }
        _ST[has_lnb] = st
    return st


def kernel(x, ln_w, ln_b, w_qkv, w_out, b_out):
    x = np.asarray(x, dtype=np.float32)
    ln_w = np.asarray(ln_w, dtype=np.float32)
    ln_b = np.asarray(ln_b, dtype=np.float32)
    w_qkv = np.asarray(w_qkv, dtype=np.float32)
    w_out = np.asarray(w_out, dtype=np.float32)
    b_out = np.asarray(b_out, dtype=np.float32)
    assert x.shape == (B, D, N)

    has_lnb = bool(np.any(ln_b != 0.0))

    # The axon tunnel intermittently drops on the first heavy launch after
    # other jax work in the same process ("worker hung up"/"mesh desynced");
    # reset the client and retry.
    last = None
    for _ in range(3):
        try:
            st = _get_state(has_lnb)
            return _run(st, x, has_lnb, ln_w, ln_b, w_qkv, w_out, b_out)
        except RuntimeError as e:
            last = e
            _reset_backend()
    raise last


def _put_weights(st, wkey, has_lnb, ln_w, ln_b, w_qkv, w_out, b_out):
    if st["wkey"] == wkey:
        return
    wq = (w_qkv[:, 0:512] * ln_w[:, None]).astype(np.float16)
    wk = w_qkv[:, 512:1024] * ln_w[:, None]
    wv = w_qkv[:, 1024:1536] * ln_w[:, None]
    wkv = np.concatenate([wk, wv], axis=1).astype(np.float16)
    host = {
        "wq": _replicate8(wq.reshape(DC, 128, 512)),
        "wkv": _replicate8(wkv.reshape(DC, 128, 1024)),
        "wout": _replicate8(w_out.astype(np.float16).reshape(DC, 128, D)),
        "bias": _replicate8(b_out.reshape(DC, 128, 1)),
    }
    if has_lnb:
        qb = (SCALE * (ln_b @ (w_qkv[:, 0:512] * ln_w[:, None]))).astype(np.float32)
        kvb = (ln_b @ np.concatenate([wk, wv], axis=1)).astype(np.float16)
        host["qb"] = _replicate8(qb.reshape(DC, 128, 1))
        host["kvb"] = _replicate8(kvb.reshape(1, 1024))
    st["devw"] = {k: jax.device_put(v, st["shard"]) for k, v in host.items()}
    for a in st["devw"].values():
        a.block_until_ready()
    st["wkey"] = wkey


def _run(st, x, has_lnb, ln_w, ln_b, w_qkv, w_out, b_out):
    res = np.empty((B, D, N), np.float32)
    out_idx = st["out_names"].index("out")

    # Both launches are dispatched up front (jax dispatch and device_put are
    # async), so launch 2's x upload pipelines behind launch 1's exec and
    # overlaps launch 1's y download on the duplex tunnel. All jax calls stay
    # on this one thread: concurrent multi-thread entry into the axon client
    # intermittently drops the tunnel ("notify failed ... worker hung up").
    # The very first launch after other jax work (e.g. a jitted reference in
    # the same process) runs fully synchronously: a cold pipelined burst on a
    # busy terminal intermittently drops the tunnel too.
    sync = not st.get("warmed", False)
    st["warmed"] = True
    ogs = []
    for launch in range(NLAUNCH):
        # quantize shard c, enqueue its H2D (async device_put, single thread),
        # then quantize c+1 while c streams up
        devices = list(st["mesh"].devices.flat)
        xg = np.empty((8 * DC, 128, LTOK), np.int8)
        xv = xg.reshape(8, DC, 128, LTOK)
        tmp = np.empty((DC, 128, LTOK), np.float32)
        parts = []
        # per-token int8 scale (127/absmax; cancels inside the device LN),
        # computed lazily per batch so shard 0's upload starts sooner
        tscs = [None] * B
        for c in range(8):
            b, q = launch * BPL + c // CPB, c % CPB
            sl = slice(q * LTOK, (q + 1) * LTOK)
            if tscs[b] is None:
                xb = x[b]
                tscs[b] = 127.0 / np.maximum(
                    np.maximum(xb.max(axis=0), -xb.min(axis=0)), 1e-30
                )
            np.multiply(x[b].reshape(DC, 128, N)[:, :, sl], tscs[b][sl], out=tmp)
            np.rint(tmp, out=tmp)
            xv[c] = tmp
            parts.append(jax.device_put(xv[c], devices[c]))
        xd = jax.make_array_from_single_device_arrays(
            (8 * DC, 128, LTOK), st["shard"], parts
        )
        if sync:
            xd.block_until_ready()
        # weight check/upload after the x shards are enqueued: the ~7 ms
        # content hash overlaps the in-flight H2D (warm calls: no-op check)
        if launch == 0:
            hsh = hashlib.blake2b(digest_size=16)
            for a in (ln_w, ln_b, w_qkv, w_out, b_out):
                hsh.update(np.ascontiguousarray(a).tobytes())
            _put_weights(st, hsh.digest(), has_lnb, ln_w, ln_b, w_qkv, w_out, b_out)
        args = []
        for name in st["in_names"]:
            args.append(xd if name == "x" else st["devw"][name])
        og = st["fn"](*args)[out_idx]
        if sync:
            og.block_until_ready()
        ogs.append(og)

    # Prefetch every output shard with copy_to_host_async (still one thread,
    # no concurrent client entry), then dequantize shard c while shards
    # c+1.. are still streaming down - hides the dequant inside the D2H.
    for launch in range(NLAUNCH):
        shards = sorted(
            ogs[launch].addressable_shards, key=lambda s: st["dev2idx"][s.device]
        )
        datas = [s.data for s in shards]
        for d in datas:
            d.copy_to_host_async()
        for c, d in enumerate(datas):
            o = np.asarray(d).reshape(DC, 128, LTOK + LNT * 4)
            sc = o[:, :, LTOK:].copy().view(np.float32)  # [DC, 128, LNT]
            y = o[:, :, :LTOK].reshape(DC, 128, LNT, TN)
            b, q = launch * BPL + c // CPB, c % CPB
            rv = res[b].reshape(DC, 128, NTOT, TN)
            np.multiply(
                y, sc[:, :, :, None],
                out=rv[:, :, q * LNT : (q + 1) * LNT, :],
            )
    return res
